# revision 11
# baseline (speedup 1.0000x reference)
"""Mixtral sparse MoE block on 8 Trainium2 NeuronCores.

Strategy: FFN-dim (expert-slice) parallel. Instead of one expert per
core (which makes every core pay the MAX expert membership, 528 slots
for the reference routing while the mean is only 479), each core owns a
512-wide slice of the FFN dimension of ALL 8 experts' weights:
  core c holds w1[:, :, 512c:512c+512], w2[:, :, 512c:512c+512],
              w3[:, 512c:512c+512, :].
Every core processes the SAME gathered token buffer (one segment per
expert, concatenated; S ~= 3848 slots for the reference routing) and
produces a partial output  y_c = sum over its F-slice.  The host sums
the 8 partials, applies the routing gates and scatter-adds into the
full [T, H] output.

Per-core compute = S * 3 * H * 512 MACs = 192*S cycles ~= 308us at
2.4GHz -- perfectly load-balanced for ANY routing (the old
expert-parallel layout had a 338us floor).  Weight DMA per core is the
same 48MB bf16; activation traffic grows to ~33MB but total DMA
(~86MB, 240us) stays under the compute floor.

All matmuls run in bf16 with fp32 PSUM accumulation.  Partial outputs
are returned in bf16; the fp32 host-side sum of 8 partials adds ~1e-3
relative error, well inside the 2e-2 gate (measured total ~4.4e-3).

Scheduling notes (from trace analysis):
- Every y-store trigger is issued from the SCALAR engine (also a HWDGE
  trigger engine), so the in-order sync engine only carries weight/
  token prefetch triggers and never blocks behind cast semaphores
  (that blocking starved the w3 prefetch and cost ~6us every 3 ht
  iterations in an earlier revision).
- Phase B packs up to 4 segment-blocks into one PSUM tile (bank-aligned
  512-col slots), cutting accumulation-group boundaries from 9/ht to
  3/ht (~160ns of PE re-ramp each).
- y rows are batched two ht at a time, 4-way partition-split stores.
- Segments with cap > 512 are split into (512, cap-512) column blocks;
  both blocks share one LDWEIGHTS (the tiny block's weights load hides
  under the big block's stream).
"""

import sys

for _p in ("/opt/trn_rl_repo", "/root/.axon_site/_ro/trn_rl_repo"):
    if _p not in sys.path:
        sys.path.append(_p)

import numpy as np

H = 2048   # hidden dim
F = 4096   # ffn dim
E = 8      # experts
HT = H // 128
FL = F // E      # per-core ffn slice = 512
FLT = FL // 128  # 4 local f tiles
MAXSEG = 1024    # max token columns per segment (2 PSUM banks)

_COMPILED = {}

# set by a driver (e.g. test.py) to profile the next dispatch
TRACE = False
LAST_EXEC_NS = None
LAST_RESULTS = None


def _ensure_ntff_hook():
    """Install antenv.axon_hooks shim + register the axon NTFF profile hook
    if the image's antenv package lacks it. Only needed for TRACE runs."""
    try:
        from antenv import axon_hooks  # noqa: F401
        return
    except ImportError:
        pass
    import types
    import antenv

    mod = types.ModuleType("antenv.axon_hooks")
    _hook = [None]
    mod.set_axon_ntff_profile_hook = lambda h: _hook.__setitem__(0, h)
    mod.get_axon_ntff_profile_hook = lambda: _hook[0]
    sys.modules["antenv.axon_hooks"] = mod
    antenv.axon_hooks = mod
    try:
        from trn_agent_boot.trn_boot import _ntff_profile_via_ctypes
        mod.set_axon_ntff_profile_hook(
            _ntff_profile_via_ctypes("/opt/axon/libaxon_pjrt.so")
        )
    except Exception:
        pass


def _blocks(cap: int):
    if cap <= 512:
        return [(0, cap)]
    assert cap <= MAXSEG
    return [(0, 512), (512, cap)]


def _build(caps: tuple):
    import concourse.bacc as bacc
    import concourse.tile as tile
    from concourse import mybir

    F32 = mybir.dt.float32
    BF16 = mybir.dt.bfloat16
    COPY = mybir.ActivationFunctionType.Copy

    nseg = len(caps)
    offs = [0]
    for c in caps:
        offs.append(offs[-1] + c)
    S = offs[-1]
    CAPMAX = max(caps)
    NSF = nseg * FLT

    # phase-B drain blocks packed into PSUM tiles (<=4 bank-aligned 512-col
    # slots each).  Groups are CONTIGUOUS column runs (so the final row can
    # store per-group ranges) with balanced column counts, so each group's
    # cast chain keeps pace with the next group's matmuls.
    blocks_all = []
    for s, cap in enumerate(caps):
        for (lo, hi) in _blocks(cap):
            blocks_all.append((s, lo, hi))
    nblk = len(blocks_all)
    ngrp = (nblk + 3) // 4
    bgroups = []
    i = 0
    cum = 0
    for g in range(ngrp):
        rem = ngrp - g
        n_min = max(1, nblk - i - 4 * (rem - 1))
        n_max = min(4, nblk - i - (rem - 1))
        best_n = max(n_min, min(n_max, round((nblk - i) / rem)))
        bgroups.append(blocks_all[i:i + best_n])
        cum += sum(hi - lo for (_, lo, hi) in blocks_all[i:i + best_n])
        i += best_n

    nc = bacc.Bacc("TRN2", target_bir_lowering=False, debug=False, num_devices=E)
    # pre-tiled layouts (see kernel() for the host-side packing):
    #   xgT[p, s*16+t, c]   = x_seg_s[c, t*128+p]
    #   w1[p, s*4+ft, t, j] = w1_slice_{e(s)}[t*128+p, ft*128+j]  (w2 same)
    #   w3[p, t, s*4+ft, j] = w3_slice_{e(s)}[ft*128+p, t*128+j]
    #   yT[p, t, c]         = y_partial[c, t*128+p]
    xgT = nc.dram_tensor("xgT", [128, nseg * HT, CAPMAX], BF16,
                         kind="ExternalInput").ap()
    w1 = nc.dram_tensor("w1", [128, NSF, HT, 128], BF16,
                        kind="ExternalInput").ap()
    w2 = nc.dram_tensor("w2", [128, NSF, HT, 128], BF16,
                        kind="ExternalInput").ap()
    w3 = nc.dram_tensor("w3", [128, HT, NSF, 128], BF16,
                        kind="ExternalInput").ap()
    yT = nc.dram_tensor("yT", [128, HT, S], BF16, kind="ExternalOutput").ap()

    with tile.TileContext(nc) as tc:
        with (
            tc.tile_pool(name="resident", bufs=1) as resident,
            tc.tile_pool(name="xgpool", bufs=3) as xgpool,
            tc.tile_pool(name="wpool", bufs=2) as wpool,
            tc.tile_pool(name="w3pool", bufs=5) as w3pool,
            tc.tile_pool(name="spool", bufs=2) as spool,
            tc.tile_pool(name="ypool", bufs=3) as ypool,
            tc.tile_pool(name="ps", bufs=2, space="PSUM") as psp,
        ):
            hT_s = resident.tile([128, FLT, S], BF16)

            # HAM warmup: dummy bf16 matmuls on a memset zeros tile run
            # while the first token/weight DMAs stream in, so the PE
            # clock-gate is released (2.4GHz) before real matmuls start.
            warm = resident.tile([128, 256], BF16)
            nc.gpsimd.memset(warm[:], 0.0)
            pw = psp.tile([128, 2048], F32, tag="ps")
            for i in range(48):
                nc.tensor.matmul(
                    pw[:, :256], warm[:, :128], warm[:], start=True, stop=True
                )

            # Phase A: hT[ft, seg] = silu(w1_ft.T @ xg_seg) * (w2_ft.T @ xg_seg)
            for s in range(nseg):
                cap = caps[s]
                bl = _blocks(cap)
                xg = xgpool.tile([128, HT, CAPMAX], BF16, tag="xg")
                if s == 0:
                    # ramp: first token quarter + first w1 chunk enable the
                    # first matmuls ASAP (DMA queues round-robin across
                    # active descriptors, so finer splits land progressively)
                    nc.sync.dma_start(xg[:, :2], xgT[:, :2])
                    w1c0 = wpool.tile([128, HT, 128], BF16, tag="w1c")
                    nc.sync.dma_start(w1c0[:, :4], w1[:, 0, :4])
                    nc.sync.dma_start(xg[:, 2:6], xgT[:, 2:6])
                    nc.sync.dma_start(w1c0[:, 4:], w1[:, 0, 4:])
                    nc.sync.dma_start(xg[:, 6:11], xgT[:, 6:11])
                    nc.sync.dma_start(xg[:, 11:], xgT[:, 11:HT])
                else:
                    nc.sync.dma_start(xg[:], xgT[:, s * HT:(s + 1) * HT])
                for ft in range(FLT):
                    sf = s * FLT + ft
                    if s == 0 and ft == 0:
                        w1c = w1c0
                    else:
                        w1c = wpool.tile([128, HT, 128], BF16, tag="w1c")
                        nc.sync.dma_start(w1c[:], w1[:, sf])
                    w2c = wpool.tile([128, HT, 128], BF16, tag="w2c")
                    nc.sync.dma_start(w2c[:], w2[:, sf])

                    ps = psp.tile([128, 2048], F32, tag="ps")
                    # w1 path at fp32 cols [0:cap], w2 path at [1024:1024+cap]
                    for t in range(HT):
                        for (lo, hi) in bl:
                            nc.tensor.matmul(
                                ps[:, lo:hi], w1c[:, t], xg[:, t, lo:hi],
                                start=(t == 0), stop=(t == HT - 1),
                            )
                    for t in range(HT):
                        for (lo, hi) in bl:
                            nc.tensor.matmul(
                                ps[:, 1024 + lo:1024 + hi], w2c[:, t],
                                xg[:, t, lo:hi],
                                start=(t == 0), stop=(t == HT - 1),
                            )
                    sa = spool.tile([128, CAPMAX], F32, tag="sa")
                    nc.scalar.activation(
                        sa[:, :cap], ps[:, :cap],
                        mybir.ActivationFunctionType.Silu,
                    )
                    nc.vector.tensor_mul(
                        hT_s[:, ft, offs[s]:offs[s] + cap],
                        sa[:, :cap], ps[:, 1024:1024 + cap],
                    )

            # Phase B: yT[t] = sum_ft w3_chunk(t,seg,ft).T @ hT[ft, seg]
            # Casts alternate vector/scalar per GROUP: each engine's strict
            # FIFO then only ever holds casts whose matmuls are already done
            # or imminent (same-engine head-of-line blocking starved the
            # PSUM rotation in an earlier revision).  All store triggers
            # live on sync; its only other phase-B work is w3 prefetch,
            # which the 5-deep w3pool cushions.
            gctr = 0
            for t in range(HT):
                w3c = w3pool.tile([128, NSF, 128], BF16, tag="w3c")
                nc.sync.dma_start(w3c[:], w3[:, t])
                yt = ypool.tile([128, S], BF16, tag="yt")
                last = (t == HT - 1)
                for g in bgroups:
                    ps = psp.tile([128, 2048], F32, tag="ps")
                    for ft in range(FLT):
                        for gi, (s, lo, hi) in enumerate(g):
                            nc.tensor.matmul(
                                ps[:, 512 * gi:512 * gi + (hi - lo)],
                                w3c[:, s * FLT + ft],
                                hT_s[:, ft, offs[s] + lo:offs[s] + hi],
                                start=(ft == 0), stop=(ft == FLT - 1),
                            )
                    for gi, (s, lo, hi) in enumerate(g):
                        dst = yt[:, offs[s] + lo:offs[s] + hi]
                        src = ps[:, 512 * gi:512 * gi + (hi - lo)]
                        if gctr % 2 == 0:
                            nc.vector.tensor_copy(dst, src)
                        else:
                            nc.scalar.activation(dst, src, COPY)
                    gctr += 1
                    if last:
                        # final row: store each group's columns right after
                        # its casts so the terminal DMA drain is tiny
                        glo = min(offs[s] + lo for (s, lo, hi) in g)
                        ghi = max(offs[s] + hi for (s, lo, hi) in g)
                        for q in range(2):
                            nc.sync.dma_start(
                                yT[64 * q:64 * q + 64, t, glo:ghi],
                                yt[64 * q:64 * q + 64, glo:ghi],
                            )
                if not last:
                    # store split across the PARTITION dim (4x queue
                    # parallelism)
                    for q in range(4):
                        nc.sync.dma_start(
                            yT[32 * q:32 * q + 32, t], yt[32 * q:32 * q + 32]
                        )

    nc.compile()
    return nc


def _get_compiled(caps: tuple):
    if caps not in _COMPILED:
        _COMPILED[caps] = _build(caps)
    return _COMPILED[caps]


def _segments(idx):
    """Split per-expert token lists into segments of <= MAXSEG tokens.
    Returns list of (expert, token_index_array, cap)."""
    segs = []
    for e in range(E):
        ii = idx[e]
        for lo in range(0, len(ii), MAXSEG):
            chunk = ii[lo:lo + MAXSEG]
            cap = max(16, ((len(chunk) + 3) // 4) * 4)
            segs.append((e, chunk, cap))
    return segs


def kernel(hidden_states, selected_experts, routing_weights, w1, w2, w3):
    global LAST_EXEC_NS, LAST_RESULTS
    from concourse.bass_utils import run_bass_kernel_spmd
    import ml_dtypes

    BF = ml_dtypes.bfloat16

    hs = np.ascontiguousarray(np.asarray(hidden_states), dtype=np.float32)
    sel = np.asarray(selected_experts)
    rw = np.ascontiguousarray(np.asarray(routing_weights), dtype=np.float32)
    w1 = np.asarray(w1)
    w2 = np.asarray(w2)
    w3 = np.asarray(w3)

    T = hs.shape[0]
    K = sel.shape[1]
    assert hs.shape[1] == H and w1.shape == (E, H, F) and w3.shape == (E, F, H)

    # host routing: gate[t, e] = sum_k rw[t, k] * (sel[t, k] == e)
    gate = np.zeros((T, E), np.float32)
    member = np.zeros((T, E), bool)
    tix = np.arange(T)
    for k in range(K):
        np.add.at(gate, (tix, sel[:, k]), rw[:, k])
        member[tix, sel[:, k]] = True
    idx = [np.nonzero(member[:, e])[0] for e in range(E)]

    segs = _segments(idx)
    nseg = len(segs)
    caps = tuple(c for (_, _, c) in segs)
    offs = np.concatenate([[0], np.cumsum(caps)]).astype(int)
    S = int(offs[-1])
    CAPMAX = max(caps)
    segexp = np.array([e for (e, _, _) in segs])

    xr = hs.astype(BF)  # [T, H]

    # gathered tokens, transposed + tiled: xgT[p, s*16+t, c] = x[seg_s[c], t*128+p]
    xgT = np.zeros((128, nseg * HT, CAPMAX), BF)
    for si, (e, ii, cap) in enumerate(segs):
        if len(ii):
            xgT[:, si * HT:(si + 1) * HT, :len(ii)] = (
                xr[ii].reshape(len(ii), HT, 128).transpose(2, 1, 0)
            )

    # per-core weight slices (bf16, per-partition-contiguous)
    w1b = w1.astype(BF)   # [E, H, F]
    w2b = w2.astype(BF)
    w3b = w3.astype(BF)   # [E, F, H]
    in_maps = []
    for c in range(E):
        sl = slice(c * FL, (c + 1) * FL)
        # [E, H, FL] -> [e, ht, p, ft, j] -> [p, e, ft, ht, j] -> seg slots
        w1p = w1b[:, :, sl].reshape(E, HT, 128, FLT, 128).transpose(2, 0, 3, 1, 4)
        w1p = np.ascontiguousarray(w1p[:, segexp]).reshape(128, nseg * FLT, HT, 128)
        w2p = w2b[:, :, sl].reshape(E, HT, 128, FLT, 128).transpose(2, 0, 3, 1, 4)
        w2p = np.ascontiguousarray(w2p[:, segexp]).reshape(128, nseg * FLT, HT, 128)
        # [E, FL, H] -> [e, ft, p, ht, j] -> [p, ht, e, ft, j] -> seg slots
        w3p = w3b[:, sl, :].reshape(E, FLT, 128, HT, 128).transpose(2, 3, 0, 1, 4)
        w3p = np.ascontiguousarray(w3p[:, :, segexp]).reshape(128, HT, nseg * FLT, 128)
        in_maps.append({"xgT": xgT, "w1": w1p, "w2": w2p, "w3": w3p})

    if TRACE:
        _ensure_ntff_hook()
    nc = _get_compiled(caps)
    res = run_bass_kernel_spmd(
        nc, in_maps, core_ids=list(range(E)),
        trace=TRACE, trace_cores=(list(range(E)) if TRACE else None),
    )
    if TRACE:
        LAST_EXEC_NS = res.exec_time_ns
        LAST_RESULTS = res

    # sum partials in fp32, un-tile, apply gates, scatter-add
    ysum = np.zeros((128, HT, S), np.float32)
    for c in range(E):
        ysum += res.results[c]["yT"].astype(np.float32)
    y = ysum.transpose(2, 1, 0).reshape(S, H)  # [S, H]
    out = np.zeros((T, H), np.float32)
    for si, (e, ii, cap) in enumerate(segs):
        if len(ii):
            out[ii] += gate[ii, e:e + 1] * y[offs[si]:offs[si] + len(ii)]
    return out


# revision 16
# speedup vs baseline: 1.0296x; 1.0296x over previous
"""Mixtral sparse MoE block on 8 Trainium2 NeuronCores.

Strategy: FFN-dim (expert-slice) parallel. Instead of one expert per
core (which makes every core pay the MAX expert membership, 528 slots
for the reference routing while the mean is only 479), each core owns a
512-wide slice of the FFN dimension of ALL 8 experts' weights:
  core c holds w1[:, :, 512c:512c+512], w2[:, :, 512c:512c+512],
              w3[:, 512c:512c+512, :].
Every core processes the SAME gathered token buffer (one segment per
expert, concatenated; S ~= 3848 slots for the reference routing) and
produces a partial output  y_c = sum over its F-slice.  The host sums
the 8 partials, applies the routing gates and scatter-adds into the
full [T, H] output.

Per-core compute = S * 3 * H * 512 MACs = 192*S cycles ~= 308us at
2.4GHz -- perfectly load-balanced for ANY routing (the old
expert-parallel layout had a 338us floor).  Weight DMA per core is the
same 48MB bf16; activation traffic grows to ~33MB but total DMA
(~86MB, 240us) stays under the compute floor.

All matmuls run in bf16 with fp32 PSUM accumulation.  Partial outputs
are returned in bf16; the fp32 host-side sum of 8 partials adds ~1e-3
relative error, well inside the 2e-2 gate (measured total ~4.4e-3).

Scheduling notes (from trace analysis):
- Every y-store trigger is issued from the SCALAR engine (also a HWDGE
  trigger engine), so the in-order sync engine only carries weight/
  token prefetch triggers and never blocks behind cast semaphores
  (that blocking starved the w3 prefetch and cost ~6us every 3 ht
  iterations in an earlier revision).
- Phase B packs up to 4 segment-blocks into one PSUM tile (bank-aligned
  512-col slots), cutting accumulation-group boundaries from 9/ht to
  3/ht (~160ns of PE re-ramp each).
- y rows are batched two ht at a time, 4-way partition-split stores.
- Segments with cap > 512 are split into (512, cap-512) column blocks;
  both blocks share one LDWEIGHTS (the tiny block's weights load hides
  under the big block's stream).
"""

import sys

for _p in ("/opt/trn_rl_repo", "/root/.axon_site/_ro/trn_rl_repo"):
    if _p not in sys.path:
        sys.path.append(_p)

import numpy as np

H = 2048   # hidden dim
F = 4096   # ffn dim
E = 8      # experts
HT = H // 128
FL = F // E      # per-core ffn slice = 512
FLT = FL // 128  # 4 local f tiles
MAXSEG = 1024    # max token columns per segment (2 PSUM banks)

_COMPILED = {}

# set by a driver (e.g. test.py) to profile the next dispatch
TRACE = False
LAST_EXEC_NS = None
LAST_RESULTS = None


def _ensure_ntff_hook():
    """Install antenv.axon_hooks shim + register the axon NTFF profile hook
    if the image's antenv package lacks it. Only needed for TRACE runs."""
    try:
        from antenv import axon_hooks  # noqa: F401
        return
    except ImportError:
        pass
    import types
    import antenv

    mod = types.ModuleType("antenv.axon_hooks")
    _hook = [None]
    mod.set_axon_ntff_profile_hook = lambda h: _hook.__setitem__(0, h)
    mod.get_axon_ntff_profile_hook = lambda: _hook[0]
    sys.modules["antenv.axon_hooks"] = mod
    antenv.axon_hooks = mod
    try:
        from trn_agent_boot.trn_boot import _ntff_profile_via_ctypes
        mod.set_axon_ntff_profile_hook(
            _ntff_profile_via_ctypes("/opt/axon/libaxon_pjrt.so")
        )
    except Exception:
        pass


def _blocks(cap: int):
    if cap <= 512:
        return [(0, cap)]
    assert cap <= MAXSEG
    return [(0, 512), (512, cap)]


def _build(caps: tuple):
    import concourse.bacc as bacc
    import concourse.tile as tile
    from concourse import mybir

    F32 = mybir.dt.float32
    BF16 = mybir.dt.bfloat16
    COPY = mybir.ActivationFunctionType.Copy

    nseg = len(caps)
    offs = [0]
    for c in caps:
        offs.append(offs[-1] + c)
    S = offs[-1]
    CAPMAX = max(caps)
    NSF = nseg * FLT

    # phase-B drain blocks packed into PSUM tiles: <=2 bank-aligned 512-col
    # slots per [128,1024] tile, contiguous column runs.  Small groups +
    # 4-deep PSUM rotation keep the PE fed even when the tile scheduler
    # interleaves independent accumulation chains.
    blocks_all = []
    for s, cap in enumerate(caps):
        for (lo, hi) in _blocks(cap):
            blocks_all.append((s, lo, hi))
    nblk = len(blocks_all)
    ngrp = (nblk + 1) // 2
    bgroups = []
    i = 0
    for g in range(ngrp):
        rem = ngrp - g
        n_min = max(1, nblk - i - 2 * (rem - 1))
        n_max = min(2, nblk - i - (rem - 1))
        best_n = max(n_min, min(n_max, round((nblk - i) / rem)))
        bgroups.append(blocks_all[i:i + best_n])
        i += best_n

    nc = bacc.Bacc("TRN2", target_bir_lowering=False, debug=False, num_devices=E)
    # pre-tiled layouts (see kernel() for the host-side packing):
    #   xgT[p, s*16+t, c]   = x_seg_s[c, t*128+p]
    #   w1[p, s*4+ft, t, j] = w1_slice_{e(s)}[t*128+p, ft*128+j]  (w2 same)
    #   w3[p, t, s*4+ft, j] = w3_slice_{e(s)}[ft*128+p, t*128+j]
    #   yT[p, t, c]         = y_partial[c, t*128+p]
    xgT = nc.dram_tensor("xgT", [128, nseg * HT, CAPMAX], BF16,
                         kind="ExternalInput").ap()
    w1 = nc.dram_tensor("w1", [128, NSF, HT, 128], BF16,
                        kind="ExternalInput").ap()
    w2 = nc.dram_tensor("w2", [128, NSF, HT, 128], BF16,
                        kind="ExternalInput").ap()
    w3 = nc.dram_tensor("w3", [128, HT, NSF, 128], BF16,
                        kind="ExternalInput").ap()
    yT = nc.dram_tensor("yT", [128, HT, S], BF16, kind="ExternalOutput").ap()

    with tile.TileContext(nc) as tc:
        with (
            tc.tile_pool(name="resident", bufs=1) as resident,
            tc.tile_pool(name="xgpool", bufs=3) as xgpool,
            tc.tile_pool(name="wpool", bufs=2) as wpool,
            tc.tile_pool(name="w3pool", bufs=5) as w3pool,
            tc.tile_pool(name="spool", bufs=2) as spool,
            tc.tile_pool(name="ypool", bufs=3) as ypool,
            tc.tile_pool(name="ps", bufs=4, space="PSUM") as psp,
        ):
            hT_s = resident.tile([128, FLT, S], BF16)

            # HAM warmup: dummy bf16 matmuls on a memset zeros tile run
            # while the first token/weight DMAs stream in, so the PE
            # clock-gate is released (2.4GHz) before real matmuls start.
            warm = resident.tile([128, 256], BF16)
            nc.gpsimd.memset(warm[:], 0.0)
            pw = psp.tile([128, 1024], F32, tag="ps")
            for i in range(48):
                nc.tensor.matmul(
                    pw[:, :256], warm[:, :128], warm[:], start=True, stop=True
                )

            # Phase A: hT[ft, seg] = silu(w1_ft.T @ xg_seg) * (w2_ft.T @ xg_seg)
            for s in range(nseg):
                cap = caps[s]
                bl = _blocks(cap)
                xg = xgpool.tile([128, HT, CAPMAX], BF16, tag="xg")
                if s == 0:
                    # ramp: first token quarter + first w1 chunk enable the
                    # first matmuls ASAP (DMA queues round-robin across
                    # active descriptors, so finer splits land progressively)
                    nc.sync.dma_start(xg[:, :2], xgT[:, :2])
                    w1c0 = wpool.tile([128, HT, 128], BF16, tag="w1c")
                    nc.sync.dma_start(w1c0[:, :4], w1[:, 0, :4])
                    nc.sync.dma_start(xg[:, 2:6], xgT[:, 2:6])
                    nc.sync.dma_start(w1c0[:, 4:], w1[:, 0, 4:])
                    nc.sync.dma_start(xg[:, 6:11], xgT[:, 6:11])
                    nc.sync.dma_start(xg[:, 11:], xgT[:, 11:HT])
                else:
                    nc.sync.dma_start(xg[:], xgT[:, s * HT:(s + 1) * HT])
                for ft in range(FLT):
                    sf = s * FLT + ft
                    if s == 0 and ft == 0:
                        w1c = w1c0
                    else:
                        w1c = wpool.tile([128, HT, 128], BF16, tag="w1c")
                        nc.sync.dma_start(w1c[:], w1[:, sf])
                    w2c = wpool.tile([128, HT, 128], BF16, tag="w2c")
                    nc.sync.dma_start(w2c[:], w2[:, sf])

                    ps1 = psp.tile([128, 1024], F32, tag="ps")
                    ps2 = psp.tile([128, 1024], F32, tag="ps")
                    for t in range(HT):
                        for (lo, hi) in bl:
                            nc.tensor.matmul(
                                ps1[:, lo:hi], w1c[:, t], xg[:, t, lo:hi],
                                start=(t == 0), stop=(t == HT - 1),
                            )
                    for t in range(HT):
                        for (lo, hi) in bl:
                            nc.tensor.matmul(
                                ps2[:, lo:hi], w2c[:, t], xg[:, t, lo:hi],
                                start=(t == 0), stop=(t == HT - 1),
                            )
                    sa = spool.tile([128, CAPMAX], F32, tag="sa")
                    nc.scalar.activation(
                        sa[:, :cap], ps1[:, :cap],
                        mybir.ActivationFunctionType.Silu,
                    )
                    nc.vector.tensor_mul(
                        hT_s[:, ft, offs[s]:offs[s] + cap],
                        sa[:, :cap], ps2[:, :cap],
                    )

            # Phase B: yT[t] = sum_ft w3_chunk(t,seg,ft).T @ hT[ft, seg]
            # Small (<=2 block) groups with 4-deep PSUM rotation keep the PE
            # fed even when the tile scheduler interleaves independent
            # accumulation chains.  Casts alternate vector/scalar per group.
            # Store triggers live on the SCALAR engine: its DGE queues are
            # separate from sync's, so bulk y stores never queue in front of
            # the latency-critical w3 prefetch (they did when both shared
            # sync's queues, starving the PE ~12us at a time).
            gctr = 0
            for t in range(HT):
                w3c = w3pool.tile([128, NSF, 128], BF16, tag="w3c")
                nc.sync.dma_start(w3c[:], w3[:, t])
                yt = ypool.tile([128, S], BF16, tag="yt")
                last = (t == HT - 1)
                for g in bgroups:
                    ps = psp.tile([128, 1024], F32, tag="ps")
                    for ft in range(FLT):
                        for gi, (s, lo, hi) in enumerate(g):
                            nc.tensor.matmul(
                                ps[:, 512 * gi:512 * gi + (hi - lo)],
                                w3c[:, s * FLT + ft],
                                hT_s[:, ft, offs[s] + lo:offs[s] + hi],
                                start=(ft == 0), stop=(ft == FLT - 1),
                            )
                    for gi, (s, lo, hi) in enumerate(g):
                        dst = yt[:, offs[s] + lo:offs[s] + hi]
                        src = ps[:, 512 * gi:512 * gi + (hi - lo)]
                        if gctr % 2 == 0:
                            nc.vector.tensor_copy(dst, src)
                        else:
                            nc.scalar.activation(dst, src, COPY)
                    gctr += 1
                    if last:
                        # final row: store each group's columns right after
                        # its casts so the terminal DMA drain is tiny
                        glo = min(offs[s] + lo for (s, lo, hi) in g)
                        ghi = max(offs[s] + hi for (s, lo, hi) in g)
                        nc.scalar.dma_start(
                            yT[:, t, glo:ghi], yt[:, glo:ghi]
                        )
                if not last:
                    # store split across the PARTITION dim (2x queue
                    # parallelism; ypool depth 3 gives stores ~2 rows of
                    # slack so latency is uncritical)
                    for q in range(2):
                        nc.scalar.dma_start(
                            yT[64 * q:64 * q + 64, t], yt[64 * q:64 * q + 64]
                        )

    nc.compile()
    return nc


def _get_compiled(caps: tuple):
    if caps not in _COMPILED:
        _COMPILED[caps] = _build(caps)
    return _COMPILED[caps]


def _segments(idx):
    """Split per-expert token lists into segments of <= MAXSEG tokens.
    Returns list of (expert, token_index_array, cap)."""
    segs = []
    for e in range(E):
        ii = idx[e]
        for lo in range(0, len(ii), MAXSEG):
            chunk = ii[lo:lo + MAXSEG]
            cap = max(16, ((len(chunk) + 3) // 4) * 4)
            segs.append((e, chunk, cap))
    return segs


def kernel(hidden_states, selected_experts, routing_weights, w1, w2, w3):
    global LAST_EXEC_NS, LAST_RESULTS
    from concourse.bass_utils import run_bass_kernel_spmd
    import ml_dtypes

    BF = ml_dtypes.bfloat16

    hs = np.ascontiguousarray(np.asarray(hidden_states), dtype=np.float32)
    sel = np.asarray(selected_experts)
    rw = np.ascontiguousarray(np.asarray(routing_weights), dtype=np.float32)
    w1 = np.asarray(w1)
    w2 = np.asarray(w2)
    w3 = np.asarray(w3)

    T = hs.shape[0]
    K = sel.shape[1]
    assert hs.shape[1] == H and w1.shape == (E, H, F) and w3.shape == (E, F, H)

    # host routing: gate[t, e] = sum_k rw[t, k] * (sel[t, k] == e)
    gate = np.zeros((T, E), np.float32)
    member = np.zeros((T, E), bool)
    tix = np.arange(T)
    for k in range(K):
        np.add.at(gate, (tix, sel[:, k]), rw[:, k])
        member[tix, sel[:, k]] = True
    idx = [np.nonzero(member[:, e])[0] for e in range(E)]

    segs = _segments(idx)
    nseg = len(segs)
    caps = tuple(c for (_, _, c) in segs)
    offs = np.concatenate([[0], np.cumsum(caps)]).astype(int)
    S = int(offs[-1])
    CAPMAX = max(caps)
    segexp = np.array([e for (e, _, _) in segs])

    xr = hs.astype(BF)  # [T, H]

    # gathered tokens, transposed + tiled: xgT[p, s*16+t, c] = x[seg_s[c], t*128+p]
    xgT = np.zeros((128, nseg * HT, CAPMAX), BF)
    for si, (e, ii, cap) in enumerate(segs):
        if len(ii):
            xgT[:, si * HT:(si + 1) * HT, :len(ii)] = (
                xr[ii].reshape(len(ii), HT, 128).transpose(2, 1, 0)
            )

    # per-core weight slices (bf16, per-partition-contiguous)
    w1b = w1.astype(BF)   # [E, H, F]
    w2b = w2.astype(BF)
    w3b = w3.astype(BF)   # [E, F, H]
    in_maps = []
    for c in range(E):
        sl = slice(c * FL, (c + 1) * FL)
        # [E, H, FL] -> [e, ht, p, ft, j] -> [p, e, ft, ht, j] -> seg slots
        w1p = w1b[:, :, sl].reshape(E, HT, 128, FLT, 128).transpose(2, 0, 3, 1, 4)
        w1p = np.ascontiguousarray(w1p[:, segexp]).reshape(128, nseg * FLT, HT, 128)
        w2p = w2b[:, :, sl].reshape(E, HT, 128, FLT, 128).transpose(2, 0, 3, 1, 4)
        w2p = np.ascontiguousarray(w2p[:, segexp]).reshape(128, nseg * FLT, HT, 128)
        # [E, FL, H] -> [e, ft, p, ht, j] -> [p, ht, e, ft, j] -> seg slots
        w3p = w3b[:, sl, :].reshape(E, FLT, 128, HT, 128).transpose(2, 3, 0, 1, 4)
        w3p = np.ascontiguousarray(w3p[:, :, segexp]).reshape(128, HT, nseg * FLT, 128)
        in_maps.append({"xgT": xgT, "w1": w1p, "w2": w2p, "w3": w3p})

    if TRACE:
        _ensure_ntff_hook()
    nc = _get_compiled(caps)
    res = run_bass_kernel_spmd(
        nc, in_maps, core_ids=list(range(E)),
        trace=TRACE, trace_cores=(list(range(E)) if TRACE else None),
    )
    if TRACE:
        LAST_EXEC_NS = res.exec_time_ns
        LAST_RESULTS = res

    # sum partials in fp32, un-tile, apply gates, scatter-add
    ysum = np.zeros((128, HT, S), np.float32)
    for c in range(E):
        ysum += res.results[c]["yT"].astype(np.float32)
    y = ysum.transpose(2, 1, 0).reshape(S, H)  # [S, H]
    out = np.zeros((T, H), np.float32)
    for si, (e, ii, cap) in enumerate(segs):
        if len(ii):
            out[ii] += gate[ii, e:e + 1] * y[offs[si]:offs[si] + len(ii)]
    return out
